# revision 1
# baseline (speedup 1.0000x reference)
"""MoE (top-2 of 8 experts, d=1024, h=4096) on 8 Trainium2 NeuronCores.

Strategy (expert-parallel, per sharding hint):
  - Host: gating (fp64 logits/softmax/top-2 — tie margins on this problem are
    ~1e-5, far above fp32 rounding noise, so host routing matches the
    reference's fp32 top-k), per-expert token gather, pad to capacity C.
  - Device (core e = expert e): hidT = relu(W1_e.T @ x_e.T + b1_e) then
    ye = hidT.T @ W2_e, both as K-tiled 128x128x512 matmuls in float32r
    (full PE rate, ~1e-4 matmul rel err).
  - Host: out[tok_e] += gate_e * (ye + b2_e)  (scatter-combine).

Self-contained: hardcodes all shapes; only imports concourse (system lib).
"""

import os

os.environ.setdefault("JAX_PLATFORMS", "")

import numpy as np

import concourse.bacc as bacc
import concourse.mybir as mybir
import concourse.tile as tile
from concourse.bass_utils import run_bass_kernel_spmd

P = 128
D = 1024  # embed dim
H = 4096  # hidden dim
E = 8  # experts
TOPK = 2
KD = D // P  # 8  k-tiles over embed
KH = H // P  # 32 k-tiles over hidden
NCORES = 8
FD = 512  # matmul moving free dim (one PSUM bank of fp32)

_compiled = {}
LAST_RESULT = None  # BassKernelResults of the most recent run (for test harness)


def _phase1(nc, tc, rs, C, chunks, xt_d, w1_d, b1_d, hid_cs):
    f32 = mybir.dt.float32
    f32r = mybir.dt.float32r
    relu = mybir.ActivationFunctionType.Relu
    TN = len(chunks)
    with (
        tc.tile_pool(name=rs + "xts_p", bufs=1) as xts_p,
        tc.tile_pool(name=rs + "b1_p", bufs=1) as b1_p,
        tc.tile_pool(name=rs + "w1_p", bufs=int(os.environ.get("MOE_W1B", "4"))) as w1_p,
        tc.tile_pool(name=rs + "hb_p", bufs=1) as hb_p,
        tc.tile_pool(name=rs + "ps1", bufs=int(os.environ.get("MOE_PS1", "4")), space="PSUM") as ps1,
    ):
        def load_w1(hm):
            w1t = w1_p.tile([P, KD, P], f32r, tag="w1t", name=rs + f"w1t_{hm}")
            nc.sync.dma_start(w1t[:], w1_d[:, hm])
            return w1t

        # Issue order matters: HWDGE dispatches in program order, so the
        # first matmul group's deps (w1t[0] + x chunk 0) are issued first.
        w1_pre = {0: load_w1(0)}
        # x chunks as separate per-k tiles so the first matmul group only
        # waits on its own 8 pieces (~2MB), not the whole 10MB load.
        xc = [[None] * KD for _ in range(TN)]
        for tn, (off, w) in enumerate(chunks):
            for k0 in range(0, KD, 2):
                t = xts_p.tile(
                    [P, 2, w], f32r, tag=f"x_{tn}_{k0}", name=rs + f"x_{tn}_{k0}"
                )
                nc.sync.dma_start(t[:], xt_d[:, k0 : k0 + 2, off : off + w])
                xc[tn][k0] = t[:, 0, :]
                xc[tn][k0 + 1] = t[:, 1, :]
            if tn == 0:
                # b1 is first needed at the first eviction, not the first
                # matmul: issue it after chunk 0's loads.
                b1s = b1_p.tile([P, KH], f32, name=rs + "b1s")
                nc.sync.dma_start(b1s[:], b1_d[:])
            if tn < 3:  # prefetch next stationary tiles early
                w1_pre[tn + 1] = load_w1(tn + 1)
        # PE emission order: the first W hm rows are swept tn-major (wave
        # order) so the earliest matmuls only touch x chunks that have
        # already landed; the rest are hm-major. Each (hm, tn) psum group is
        # independent, so this only reorders work.
        W = int(os.environ.get("MOE_W", "2")) if TN > 1 else 0
        sched = [(hm, tn) for tn in range(TN) for hm in range(W)]
        sched += [(hm, tn) for hm in range(W, KH) for tn in range(TN)]

        w1ts, done = {}, {}
        KQ1 = KH // 4
        for hm, tn in sched:
            if hm not in w1ts:
                w1ts[hm] = w1_pre.pop(hm) if hm in w1_pre else load_w1(hm)
                done[hm] = 0
            off, w = chunks[tn]
            pt = ps1.tile([P, FD], f32, tag="ps1", name=rs + f"ps1_{hm}_{tn}")
            for k in range(KD):
                nc.tensor.matmul(
                    pt[:, :w],
                    w1ts[hm][:, k, :],
                    xc[tn][k],
                    start=(k == 0),
                    stop=(k == KD - 1),
                )
            # evict through a small per-chunk staging tile (ACT does
            # relu+bias, then the hid write DMAs it straight out on the ACT
            # HWDGE ring so phase-2 loads (SP ring) aren't queued behind it)
            hbst = int(os.environ.get("MOE_HBST", "12")) if C <= 2560 else 6
            hb = hb_p.tile([P, w], f32r, tag="hbst", bufs=hbst, name=rs + f"hb_{hm}_{tn}")
            nc.scalar.activation(
                hb[:, :w], pt[:, :w], relu, bias=b1s[:, hm : hm + 1]
            )
            nc.scalar.dma_start(
                hid_cs[tn][hm // KQ1][:, :, hm % KQ1, :].transpose([1, 0, 2]),
                hb.rearrange("p (t q) -> p t q", q=P),
            )
            done[hm] += 1
            if done[hm] == TN:
                del w1ts[hm]  # release references; pool slots recycle


W2HEAD = 8  # w2 chunks living in the persistent pool (loadable during phase 1)


def _phase2(nc, tc, rs, C, chunks, w2_d, hid_cs, ye_d, hd_p, w2h_p, ps2):
    f32 = mybir.dt.float32
    f32r = mybir.dt.float32r
    TM = C // P
    with (
        tc.tile_pool(name=rs + "w2_p", bufs=1) as w2_p,
        tc.tile_pool(name=rs + "out_p", bufs=int(os.environ.get("MOE_OUTB", "3"))) as out_p,
    ):

        HDS = 4  # hd k-split (must match the 4-way hid_cs DRAM split)
        KQ = KH // HDS

        def load_hd(tm):
            cidx = next(
                i for i, (off, w) in enumerate(chunks) if off // P <= tm < (off + w) // P
            )
            local = tm - chunks[cidx][0] // P
            parts = []
            for q in range(HDS):
                hdq = hd_p.tile(
                    [P, KQ, P], f32r, tag=f"hd{q}", name=rs + f"hd_{tm}_{q}"
                )
                nc.sync.dma_start(hdq[:], hid_cs[cidx][q][local])
                parts.append(hdq)
            return parts

        # Issue order: w2 head + first token tile's data before the bulk w2
        # load, so the first phase-2 matmul isn't queued behind 16MB of w2 on
        # the in-order HWDGE ring. Head w2 + hd live in pools hoisted outside
        # phase 1's, so these loads can run during phase 1's tail.
        w2ts = []
        for k in range(W2HEAD):
            w2t = w2h_p.tile([P, D], f32r, tag=f"w2_{k}", name=rs + f"w2_{k}")
            nc.sync.dma_start(w2t[:], w2_d[k])
            w2ts.append(w2t)
        hd_pre = {0: load_hd(0)}
        for k in range(W2HEAD, KH):
            w2t = w2_p.tile([P, D], f32r, tag=f"w2_{k}", name=rs + f"w2_{k}")
            nc.sync.dma_start(w2t[:], w2_d[k])
            w2ts.append(w2t)
            if k == 15:
                hd_pre[1] = load_hd(1)
        hd_pre[2] = load_hd(2)
        for tm in range(TM):
            hd = hd_pre.pop(tm) if tm in hd_pre else load_hd(tm)
            ob = out_p.tile([P, D], f32, tag="ob", name=rs + f"ob_{tm}")
            for n in range(D // FD):
                pt2 = ps2.tile([P, FD], f32, tag="ps2", name=rs + f"ps2_{tm}_{n}")
                for k in range(KH):
                    nc.tensor.matmul(
                        pt2[:],
                        hd[k // KQ][:, k % KQ, :],
                        w2ts[k][:, n * FD : (n + 1) * FD],
                        start=(k == 0),
                        stop=(k == KH - 1),
                    )
                nc.vector.tensor_copy(ob[:, n * FD : (n + 1) * FD], pt2[:])
            nc.scalar.dma_start(ye_d[tm], ob[:])


def _build(C, reps=1):
    """Per-core SPMD program for capacity-C tokens through one expert.

    reps>1 repeats the whole program back-to-back (timing experiments only).
    """
    if (C, reps) in _compiled:
        return _compiled[(C, reps)]
    f32 = mybir.dt.float32
    f32r = mybir.dt.float32r
    TM = C // P  # token tiles (GEMM2 stationary / output rows)
    # GEMM1 moving chunks: 512s plus one remainder (multiple of 128; N>=256
    # keeps fp32r at full rate, a 128 tail is negligible)
    chunks = []
    off = 0
    CW = int(os.environ.get("MOE_CW", "0"))
    if CW and C % CW == 0:  # uniform chunk-width experiment knob
        while off < C:
            chunks.append((off, CW))
            off += CW
    else:
        if C >= 768:  # small first chunk -> first matmul group starts sooner
            chunks.append((0, 256))
            off = 256
        while off < C:
            w = min(FD, C - off)
            chunks.append((off, w))
            off += w

    nc = bacc.Bacc(None, target_bir_lowering=False)
    # xt host layout [P, KD, C]: xt[p, k, c] = x[tok_c, k*128+p] (transposed)
    xt_d = nc.dram_tensor("xt", [P, KD, C], f32r, kind="ExternalInput")
    # w1 host layout [P, KH, KD, P]: w1[p, hm, k, j] = W1[k*128+p, hm*128+j]
    # -> per-hm stationary-tile loads are contiguous 4KB per partition.
    w1_d = nc.dram_tensor("w1", [P, KH, KD, P], f32r, kind="ExternalInput")
    b1_d = nc.dram_tensor("b1", [P, KH], f32, kind="ExternalInput")
    w2_d = nc.dram_tensor("w2", [KH, P, D], f32r, kind="ExternalInput")
    ye_d = nc.dram_tensor("ye", [TM, P, D], f32, kind="ExternalOutput")

    with tile.TileContext(nc) as tc:
        with tc.tile_pool(name="dram", bufs=1, space="DRAM") as dram:
            # hidT blocks: [token-tile, hidden-in-tile (partition), hm,
            # token-in-tile] -> phase-2 reads are contiguous 16KB/partition.
            # One DRAM tile per token chunk so phase-2's first loads only
            # depend on writes to their own chunk.
            # ... and per k-quarter, so phase-2's early hd quarters depend
            # only on the phase-1 rows that produced them (DRAM deps are
            # whole-tile).
            hid_cs = [
                [
                    dram.tile(
                        [w // P, P, KH // 4, P],
                        f32r,
                        tag=f"hidc_{i}_{q}",
                        name=f"hidc_{i}_{q}",
                    )
                    for q in range(4)
                ]
                for i, (off, w) in enumerate(chunks)
            ]
            for rep in range(reps):
                rs = "" if rep == 0 else f"r{rep}_"
                # hd/w2-head/psum2 pools are hoisted outside phase 1's pools
                # so phase 2's first loads don't wait for phase-1 SBUF release.
                with (
                    tc.tile_pool(name=rs + "hd_p", bufs=3) as hd_p,
                    tc.tile_pool(name=rs + "w2h_p", bufs=1) as w2h_p,
                    tc.tile_pool(name=rs + "ps2", bufs=int(os.environ.get("MOE_PS2", "4")), space="PSUM") as ps2,
                ):
                    _phase1(nc, tc, rs, C, chunks, xt_d, w1_d, b1_d, hid_cs)
                    _phase2(
                        nc, tc, rs, C, chunks, w2_d, hid_cs, ye_d, hd_p, w2h_p, ps2
                    )

    nc.compile()
    _compiled[(C, reps)] = nc
    return nc


def kernel(x, Wg, bg, W1, b1, W2, b2):
    global LAST_RESULT
    x = np.ascontiguousarray(x, dtype=np.float32)
    B, S, d = x.shape
    assert d == D
    T = B * S
    xf = x.reshape(T, d)

    # ---- Host gating/routing (fp64) ----
    logits = xf.astype(np.float64) @ Wg.astype(np.float64) + bg.astype(np.float64)
    mx = logits.max(axis=1, keepdims=True)
    ex = np.exp(logits - mx)
    probs = ex / ex.sum(axis=1, keepdims=True)
    order = np.argsort(-logits, axis=1, kind="stable")  # ties -> lower index
    top = order[:, :TOPK]  # [T, 2]
    gsel = np.take_along_axis(probs, top, axis=1).astype(np.float32)

    toks, gates = [], []
    for e in range(E):
        pos = top == e  # [T, 2]
        sel = pos.any(axis=1)
        toks.append(np.nonzero(sel)[0])
        gates.append((gsel * pos).sum(axis=1)[sel].astype(np.float32))

    maxcnt = max(len(t) for t in toks)
    # SBUF budget caps resident x at 4096 tokens/core; batch if routing is
    # ever concentrated enough to exceed that (never for balanced gating).
    MAXC = 2944
    nb = max(1, -(-maxcnt // MAXC))
    C = max(P, ((-(-maxcnt // nb) + P - 1) // P) * P)

    w_maps = []  # per-expert weight shards (batch-invariant)
    for e in range(E):
        w_maps.append(
            {
                "w1": np.ascontiguousarray(
                    np.asarray(W1[e], dtype=np.float32)
                    .reshape(KD, P, KH, P)
                    .transpose(1, 2, 0, 3)
                ),
                "b1": np.ascontiguousarray(
                    np.asarray(b1[e], dtype=np.float32).reshape(KH, P).T
                ),
                "w2": np.ascontiguousarray(W2[e], dtype=np.float32).reshape(KH, P, D),
            }
        )

    nc = _build(C)
    out = np.zeros((T, D), np.float32)
    b2f = np.asarray(b2, dtype=np.float32)
    for b in range(nb):
        in_maps = []
        btoks = []
        for e in range(E):
            tk = toks[e][b * C : (b + 1) * C]
            btoks.append(tk)
            xe = np.zeros((C, D), np.float32)
            xe[: len(tk)] = xf[tk]
            in_maps.append(
                {
                    "xt": np.ascontiguousarray(
                        xe.T.reshape(KD, P, C).transpose(1, 0, 2)
                    ),
                    **w_maps[e],
                }
            )
        res = run_bass_kernel_spmd(nc, in_maps, core_ids=list(range(NCORES)))
        LAST_RESULT = res
        for e in range(E):
            cnt = len(btoks[e])
            if cnt == 0:
                continue
            ye = res.results[e]["ye"].reshape(C, D)[:cnt]
            g = gates[e][b * C : b * C + cnt]
            out[btoks[e]] += g[:, None] * (ye + b2f[e])
    return out.reshape(B, S, D)



# revision 2
# speedup vs baseline: 1.3730x; 1.3730x over previous
"""MoE (top-2 of 8 experts, d=1024, h=4096) on 8 Trainium2 NeuronCores.

Work split (hidden-dim parallel, perfectly balanced): every core processes
ALL experts' routed tokens, but only KSLOT=4 of the 32 hidden-dim tiles (hm)
of each expert — core c owns hm in [4c, 4c+4). Each core computes
  hid[hm-slice] = relu(x @ W1[:, hm-slice] + b1[hm-slice])
  partial_ye    = hid[hm-slice] @ W2[hm-slice, :]
and the host sums the 8 partial_ye. Per-core PE work is exactly 1/8 of the
total FLOPs regardless of routing balance (no capacity padding at all).

Numerics (fp8 hi/lo DoubleRow): every operand is split v = hi + lo with
hi = e4m3(v), lo = e5m2(v - hi) (~3.3e-4 joint representation error).
Products keep 3 of 4 terms: v·w ~= hi·hi + lo·hi + hi·lo, each computed with
fp8 DoubleRow matmuls (2 k-tiles contracted per pass at 0.5 cycles/row), so
a full contraction costs 6/8 of the bf16-matmul time at ~3e-3 overall rel
err (gate is 2e-2). Gating/top-2/combine run on host in fp64/fp32.

Self-contained: hardcodes all shapes; only imports concourse (system lib).
"""

import os

os.environ.setdefault("JAX_PLATFORMS", "")

import ml_dtypes
import numpy as np

import concourse.bacc as bacc
import concourse.mybir as mybir
import concourse.tile as tile
from concourse.bass_utils import run_bass_kernel_spmd

P = 128
D = 1024  # embed dim
H = 4096  # hidden dim
E = 8  # experts
TOPK = 2
KD = D // P  # 8  k-tiles over embed
KH = H // P  # 32 hm-tiles over hidden
NCORES = 8
KSLOT = KH // NCORES  # 4 hm-tiles per (core, expert)
DM = D // P  # 8 output d-tiles
FD = 512  # token chunk width (one PSUM bank of fp32)

E4 = ml_dtypes.float8_e4m3
E5 = ml_dtypes.float8_e5m2
DRM = mybir.MatmulPerfMode.DoubleRow

_compiled = {}
LAST_RESULT = None  # BassKernelResults of the most recent run (for test harness)


def _chunks_of(n):
    out = []
    off = 0
    while off < n:
        w = min(FD, n - off)
        out.append((off, w))
        off += w
    return out


def _build(counts):
    """Uniform per-core SPMD program; core identity comes only from inputs."""
    key = (tuple(counts), 1)
    if key in _compiled:
        return _compiled[key]
    TT = int(sum(counts))
    f32 = mybir.dt.float32
    f8e4 = mybir.dt.float8e4
    f8e5 = mybir.dt.float8e5
    bf16 = mybir.dt.bfloat16
    relu = mybir.ActivationFunctionType.Relu

    nc = bacc.Bacc(None, target_bir_lowering=False)
    xh_d = nc.dram_tensor("xh", [P, KD, TT], f8e4, kind="ExternalInput")
    xl_d = nc.dram_tensor("xl", [P, KD, TT], f8e5, kind="ExternalInput")
    w1h_d = nc.dram_tensor("w1h", [E, P, KSLOT, KD, P], f8e4, kind="ExternalInput")
    w1l_d = nc.dram_tensor("w1l", [E, P, KSLOT, KD, P], f8e5, kind="ExternalInput")
    w2h_d = nc.dram_tensor("w2h", [E, P, KSLOT, D], f8e4, kind="ExternalInput")
    w2l_d = nc.dram_tensor("w2l", [E, P, KSLOT, D], f8e5, kind="ExternalInput")
    b1_d = nc.dram_tensor("b1", [E, P, KSLOT], f32, kind="ExternalInput")
    ye_d = nc.dram_tensor("ye", [DM, P, TT], bf16, kind="ExternalOutput")

    # global (expert, chunk) work list; goff = offset in the gathered stream
    offs = np.concatenate([[0], np.cumsum(counts)]).astype(int)
    work = []  # (e, goff, w)
    for e in range(E):
        for off, w in _chunks_of(int(counts[e])):
            work.append((e, int(offs[e]) + off, w))
    elist = [e for e in range(E) if counts[e] > 0]

    with tile.TileContext(nc) as tc:
        with (
            tc.tile_pool(name="wp", bufs=3) as wp,
            tc.tile_pool(name="xp", bufs=3) as xp,
            tc.tile_pool(name="hp", bufs=3) as hp,
            tc.tile_pool(name="tp", bufs=4) as tp,
            tc.tile_pool(name="op", bufs=6) as op,
            tc.tile_pool(name="ps1", bufs=3, space="PSUM") as ps1,
            tc.tile_pool(name="ps2", bufs=5, space="PSUM") as ps2,
        ):

            def load_w1(e):
                w1h = wp.tile([P, KSLOT, KD, P], f8e4, tag="w1h", name=f"w1h_{e}")
                nc.sync.dma_start(w1h[:], w1h_d[e])
                w1l = wp.tile([P, KSLOT, KD, P], f8e5, tag="w1l", name=f"w1l_{e}")
                nc.sync.dma_start(w1l[:], w1l_d[e])
                return w1h, w1l

            def load_w2(e):
                b1s = wp.tile([P, KSLOT], f32, tag="b1", name=f"b1_{e}")
                nc.sync.dma_start(b1s[:], b1_d[e])
                w2h = wp.tile([P, KSLOT, D], f8e4, tag="w2h", name=f"w2h_{e}")
                nc.sync.dma_start(w2h[:], w2h_d[e])
                w2l = wp.tile([P, KSLOT, D], f8e5, tag="w2l", name=f"w2l_{e}")
                nc.sync.dma_start(w2l[:], w2l_d[e])
                return b1s, w2h, w2l

            def load_x(ci, goff, cw):
                xhc = xp.tile([P, KD, FD], f8e4, tag="xh", name=f"xh_{ci}")
                nc.sync.dma_start(xhc[:, :, :cw], xh_d[:, :, goff : goff + cw])
                xlc = xp.tile([P, KD, FD], f8e5, tag="xl", name=f"xl_{ci}")
                nc.sync.dma_start(xlc[:, :, :cw], xl_d[:, :, goff : goff + cw])
                return xhc, xlc

            # Issue order on the SP ring is chosen so the first G1 matmul's
            # deps (w1 of the first expert + first x chunk) land first.
            wts = {}
            e0 = elist[0]
            w1h0, w1l0 = load_w1(e0)
            x_pre = {0: load_x(0, work[0][1], work[0][2])}
            wts[e0] = (w1h0, w1l0) + load_w2(e0)

            def g1(ci, e, cw, xhc, xlc):
                w1h, w1l, b1s = wts[e][0], wts[e][1], wts[e][2]
                hh = hp.tile([P, KSLOT, FD], f8e4, tag="hh", name=f"hh_{ci}")
                hl = hp.tile([P, KSLOT, FD], f8e5, tag="hl", name=f"hl_{ci}")
                for j in range(KSLOT):
                    pt = ps1.tile([P, FD], f32, tag="ps1", name=f"ps1_{ci}_{j}")
                    n = 0
                    for src_w, src_x in ((w1h, xhc), (w1l, xhc), (w1h, xlc)):
                        for kp in range(0, KD, 2):
                            nc.tensor.matmul(
                                pt[:, :cw],
                                src_w[:, j, kp : kp + 2, :],
                                src_x[:, kp : kp + 2, :cw],
                                start=(n == 0),
                                stop=(n == 11),
                                perf_mode=DRM,
                            )
                            n += 1
                    ts_t = tp.tile([P, FD], f32, tag="ts", name=f"ts_{ci}_{j}")
                    nc.scalar.activation(
                        ts_t[:, :cw], pt[:, :cw], relu, bias=b1s[:, j : j + 1]
                    )
                    nc.gpsimd.tensor_copy(hh[:, j, :cw], ts_t[:, :cw])
                    nc.vector.scalar_tensor_tensor(
                        hl[:, j, :cw],
                        ts_t[:, :cw],
                        1.0,
                        hh[:, j, :cw],
                        mybir.AluOpType.mult,
                        mybir.AluOpType.subtract,
                    )
                return hh, hl

            def g2(ci, e, goff, cw, hh, hl):
                w2h, w2l = wts[e][3], wts[e][4]
                for dm in range(DM):
                    pt2 = ps2.tile([P, FD], f32, tag="ps2", name=f"ps2_{ci}_{dm}")
                    dms = slice(dm * P, (dm + 1) * P)
                    n = 0
                    for src_w, src_h in ((w2h, hh), (w2l, hh), (w2h, hl)):
                        for kp in range(0, KSLOT, 2):
                            nc.tensor.matmul(
                                pt2[:, :cw],
                                src_w[:, kp : kp + 2, dms],
                                src_h[:, kp : kp + 2, :cw],
                                start=(n == 0),
                                stop=(n == 5),
                                perf_mode=DRM,
                            )
                            n += 1
                    ob = op.tile([P, FD], mybir.dt.bfloat16, tag="ob", name=f"ob_{ci}_{dm}")
                    nc.vector.tensor_copy(ob[:, :cw], pt2[:, :cw])
                    nc.scalar.dma_start(ye_d[dm, :, goff : goff + cw], ob[:, :cw])

            # Software pipeline: emit G1(i) then G2(i-1), so the hid hi/lo
            # eviction chain (ACT -> gpsimd -> DVE) of chunk i completes while
            # the PE runs G1(i+1); G2(i) then starts with all inputs ready.
            prev = None  # (ci, e, goff, cw, hh, hl)
            for ci, (e, goff, cw) in enumerate(work):
                xhc, xlc = x_pre.pop(ci) if ci in x_pre else load_x(ci, goff, cw)
                # prefetch next chunk's x and (at this expert's 2nd chunk or
                # at its last chunk) the next expert's weights
                if ci + 1 < len(work):
                    ne, ngoff, ncw = work[ci + 1]
                    x_pre[ci + 1] = load_x(ci + 1, ngoff, ncw)
                    if ne not in wts:
                        wts[ne] = load_w1(ne) + load_w2(ne)
                hh, hl = g1(ci, e, cw, xhc, xlc)
                if prev is not None:
                    g2(*prev)
                prev = (ci, e, goff, cw, hh, hl)
            g2(*prev)

    nc.compile()
    _compiled[key] = nc
    return nc


def _prep_weights(W1, b1, W2):
    """Quantize + relayout weights; returns per-core input maps (cached)."""
    W1 = np.asarray(W1, dtype=np.float32)
    W2 = np.asarray(W2, dtype=np.float32)
    b1 = np.asarray(b1, dtype=np.float32)
    # [E, P(dpart), KH, KD, P(hcol)]
    w1t = np.ascontiguousarray(
        W1.reshape(E, KD, P, KH, P).transpose(0, 2, 3, 1, 4)
    )
    w1h = w1t.astype(E4)
    w1l = (w1t - w1h.astype(np.float32)).astype(E5)
    # [E, P(hpart), KH, D]
    w2t = np.ascontiguousarray(W2.reshape(E, KH, P, D).transpose(0, 2, 1, 3))
    w2h = w2t.astype(E4)
    w2l = (w2t - w2h.astype(np.float32)).astype(E5)
    # [E, P, KH]
    b1t = np.ascontiguousarray(b1.reshape(E, KH, P).transpose(0, 2, 1))
    per_core = []
    for c in range(NCORES):
        js = slice(KSLOT * c, KSLOT * (c + 1))
        per_core.append(
            {
                "w1h": np.ascontiguousarray(w1h[:, :, js]),
                "w1l": np.ascontiguousarray(w1l[:, :, js]),
                "w2h": np.ascontiguousarray(w2h[:, :, js]),
                "w2l": np.ascontiguousarray(w2l[:, :, js]),
                "b1": np.ascontiguousarray(b1t[:, :, js]),
            }
        )
    return per_core


def kernel(x, Wg, bg, W1, b1, W2, b2):
    global LAST_RESULT
    x = np.ascontiguousarray(x, dtype=np.float32)
    B, S, d = x.shape
    assert d == D
    T = B * S
    xf = x.reshape(T, d)

    # ---- Host gating/routing (fp64; tie margins far above fp32 noise) ----
    logits = xf.astype(np.float64) @ Wg.astype(np.float64) + bg.astype(np.float64)
    mx = logits.max(axis=1, keepdims=True)
    ex = np.exp(logits - mx)
    probs = ex / ex.sum(axis=1, keepdims=True)
    order = np.argsort(-logits, axis=1, kind="stable")  # ties -> lower index
    top = order[:, :TOPK]  # [T, 2]
    gsel = np.take_along_axis(probs, top, axis=1).astype(np.float32)

    toks, gates = [], []
    for e in range(E):
        pos = top == e  # [T, 2]
        sel = pos.any(axis=1)
        toks.append(np.nonzero(sel)[0])
        gates.append((gsel * pos).sum(axis=1)[sel].astype(np.float32))
    counts = [len(t) for t in toks]
    TT = int(sum(counts))

    # ---- Gather + quantize x (hi/lo fp8), transposed [P, KD, TT] ----
    xg = xf[np.concatenate(toks)] if TT else np.zeros((0, D), np.float32)
    xt = np.ascontiguousarray(xg.T.reshape(KD, P, TT).transpose(1, 0, 2))
    xh = xt.astype(E4)
    xl = (xt - xh.astype(np.float32)).astype(E5)

    w_maps = _prep_weights(W1, b1, W2)
    nc = _build(counts)

    in_maps = [{"xh": xh, "xl": xl, **w_maps[c]} for c in range(NCORES)]
    res = run_bass_kernel_spmd(nc, in_maps, core_ids=list(range(NCORES)))
    LAST_RESULT = res

    # ---- Combine: sum per-core partials, apply gates + b2, scatter ----
    acc = np.zeros((DM, P, TT), np.float32)
    for c in range(NCORES):
        acc += res.results[c]["ye"].astype(np.float32)
    ye = acc.reshape(D, TT)  # ye[d, t]
    b2f = np.asarray(b2, dtype=np.float32)
    out = np.zeros((T, D), np.float32)
    for e in range(E):
        if counts[e] == 0:
            continue
        blk = ye[:, offs_e(counts, e) : offs_e(counts, e) + counts[e]].T
        out[toks[e]] += gates[e][:, None] * (blk + b2f[e])
    return out.reshape(B, S, D)


def offs_e(counts, e):
    return int(np.sum(counts[:e]))


# revision 23
# speedup vs baseline: 1.5400x; 1.1217x over previous
"""MoE (top-2 of 8 experts, d=1024, h=4096) on 8 Trainium2 NeuronCores.

Work split (hidden-dim parallel, perfectly balanced): every core processes
ALL experts' routed tokens, but only KSLOT=4 of the 32 hidden-dim tiles (hm)
of each expert — core c owns hm in [4c, 4c+4). Each core computes
  hid[hm-slice] = relu(x @ W1[:, hm-slice] + b1[hm-slice])
  partial_ye    = hid[hm-slice] @ W2[hm-slice, :]
and the host sums the 8 partial_ye. Per-core PE work is exactly 1/8 of the
total FLOPs regardless of routing balance (no capacity padding at all).

Numerics (fp8 hi/lo DoubleRow): every operand is split v = hi + lo with
hi = e4m3(v), lo = e5m2(v - hi) (~3.3e-4 joint representation error).
Products keep 3 of 4 terms: v·w ~= hi·hi + lo·hi + hi·lo, each computed with
fp8 DoubleRow matmuls (2 k-tiles contracted per pass at 0.5 cycles/row), so
a full contraction costs 6/8 of the bf16-matmul time at ~3e-3 overall rel
err (gate is 2e-2). Gating/top-2/combine run on host in fp64/fp32.

Self-contained: hardcodes all shapes; only imports concourse (system lib).
"""

import os

os.environ.setdefault("JAX_PLATFORMS", "")

import ml_dtypes
import numpy as np

import concourse.bacc as bacc
import concourse.mybir as mybir
import concourse.tile as tile
from concourse.bass_utils import run_bass_kernel_spmd

P = 128
D = 1024  # embed dim
H = 4096  # hidden dim
E = 8  # experts
TOPK = 2
KD = D // P  # 8  k-tiles over embed
KH = H // P  # 32 hm-tiles over hidden
NCORES = 8
KSLOT = KH // NCORES  # 4 hm-tiles per (core, expert)
DM = D // P  # 8 output d-tiles
FD = 512  # token chunk width (one PSUM bank of fp32)

E4 = ml_dtypes.float8_e4m3
E5 = ml_dtypes.float8_e5m2
DRM = mybir.MatmulPerfMode.DoubleRow

_compiled = {}
LAST_RESULT = None  # BassKernelResults of the most recent run (for test harness)


def _chunks_of(n):
    out = []
    off = 0
    while off < n:
        w = min(FD, n - off)
        out.append((off, w))
        off += w
    return out


def _build(counts):
    """Uniform per-core SPMD program; core identity comes only from inputs."""
    key = (tuple(counts), 1)
    if key in _compiled:
        return _compiled[key]
    TT = int(sum(counts))
    f32 = mybir.dt.float32
    f8e4 = mybir.dt.float8e4
    f8e5 = mybir.dt.float8e5
    bf16 = mybir.dt.bfloat16
    relu = mybir.ActivationFunctionType.Relu

    nc = bacc.Bacc(None, target_bir_lowering=False)
    xh_d = nc.dram_tensor("xh", [P, KD, TT], f8e4, kind="ExternalInput")
    xl_d = nc.dram_tensor("xl", [P, KD, TT], f8e5, kind="ExternalInput")
    w1h_d = nc.dram_tensor("w1h", [E, P, KSLOT, KD, P], f8e4, kind="ExternalInput")
    w1l_d = nc.dram_tensor("w1l", [E, P, KSLOT, KD, P], f8e5, kind="ExternalInput")
    w2h_d = nc.dram_tensor("w2h", [E, P, KSLOT, D], f8e4, kind="ExternalInput")
    w2l_d = nc.dram_tensor("w2l", [E, P, KSLOT, D], f8e5, kind="ExternalInput")
    b1_d = nc.dram_tensor("b1", [E, P, KSLOT], f32, kind="ExternalInput")
    ye_d = nc.dram_tensor("ye", [DM, P, TT], bf16, kind="ExternalOutput")

    # global (expert, chunk) work list; goff = offset in the gathered stream.
    # Expert order is free (outputs are offset-addressed): put the expert
    # with the smallest tail chunk last to minimize the end-of-program drain.
    offs = np.concatenate([[0], np.cumsum(counts)]).astype(int)
    elist = [e for e in range(E) if counts[e] > 0]
    if len(elist) > 1:
        tail_w = {e: (counts[e] - 1) % FD + 1 for e in elist}
        last = min(elist, key=lambda e: tail_w[e])
        elist = [e for e in elist if e != last] + [last]
    work = []  # (e, goff, w)
    for ei, e in enumerate(elist):
        ch = _chunks_of(int(counts[e]))
        if ei == 0 and ch[0][1] > 256:
            # small first chunk so the pipeline fills fast
            ch = [(0, 128), (128, ch[0][1] - 128)] + ch[1:]
        for off, w in ch:
            work.append((e, int(offs[e]) + off, w))

    with tile.TileContext(nc) as tc:
        with (
            tc.tile_pool(name="wp", bufs=3) as wp,
            tc.tile_pool(name="xp", bufs=3) as xp,
            tc.tile_pool(name="hp", bufs=3) as hp,
            tc.tile_pool(name="tp", bufs=4) as tp,
            tc.tile_pool(name="op", bufs=3) as op,
            tc.tile_pool(name="ps1", bufs=3, space="PSUM") as ps1,
            tc.tile_pool(name="ps2", bufs=5, space="PSUM") as ps2,
        ):

            def load_w1(e, eng=None):
                # weight loads ride the ACT queue: it only issues these, so
                # pushes never block behind out-writes (which go via SP)
                eng = eng or nc.scalar
                w1h = wp.tile([P, KSLOT, KD, P], f8e4, tag="w1h", name=f"w1h_{e}")
                w1l = wp.tile([P, KSLOT, KD, P], f8e5, tag="w1l", name=f"w1l_{e}")
                eng.dma_start(w1h[:], w1h_d[e])
                eng.dma_start(w1l[:], w1l_d[e])
                return w1h, w1l

            def load_w2(e):
                b1s = wp.tile([P, KSLOT], f32, tag="b1", name=f"b1_{e}")
                nc.scalar.dma_start(b1s[:], b1_d[e])
                w2h = wp.tile([P, KSLOT, D], f8e4, tag="w2h", name=f"w2h_{e}")
                nc.scalar.dma_start(w2h[:], w2h_d[e])
                w2l = wp.tile([P, KSLOT, D], f8e5, tag="w2l", name=f"w2l_{e}")
                nc.scalar.dma_start(w2l[:], w2l_d[e])
                return b1s, w2h, w2l

            def load_x(ci, goff, cw):
                xhc = xp.tile([P, KD, FD], f8e4, tag="xh", name=f"xh_{ci}")
                nc.sync.dma_start(xhc[:, :, :cw], xh_d[:, :, goff : goff + cw])
                xlc = xp.tile([P, KD, FD], f8e5, tag="xl", name=f"xl_{ci}")
                nc.sync.dma_start(xlc[:, :, :cw], xl_d[:, :, goff : goff + cw])
                return xhc, xlc

            # Issue order on the SP ring is chosen so the first G1 matmul's
            # deps (w1 of the first expert + first x chunk) land first.
            # Startup: SP ring carries w1(e0) interleaved with chunk 0's x
            # (ACT's queue is busy with its activation-table load at t=0);
            # the rest of e0's weights follow on the ACT ring.
            wts = {}
            e0 = elist[0]
            w1h0, w1l0 = load_w1(e0)
            x_pre = {0: load_x(0, work[0][1], work[0][2])}
            wts[e0] = (w1h0, w1l0) + load_w2(e0)

            def g1(ci, e, cw, xhc, xlc):
                w1h, w1l, b1s = wts[e][0], wts[e][1], wts[e][2]
                hh = hp.tile([P, KSLOT, FD], f8e4, tag="hh", name=f"hh_{ci}")
                hl = hp.tile([P, KSLOT, FD], f8e5, tag="hl", name=f"hl_{ci}")
                for j in range(KSLOT):
                    pt = ps1.tile([P, FD], f32, tag="ps1", name=f"ps1_{ci}_{j}")
                    n = 0
                    for src_w, src_x in ((w1h, xhc), (w1l, xhc), (w1h, xlc)):
                        for kp in range(0, KD, 2):
                            nc.tensor.matmul(
                                pt[:, :cw],
                                src_w[:, j, kp : kp + 2, :],
                                src_x[:, kp : kp + 2, :cw],
                                start=(n == 0),
                                stop=(n == 11),
                                perf_mode=DRM,
                            )
                            n += 1
                    ts_t = tp.tile([P, FD], f32, tag="ts", name=f"ts_{ci}_{j}")
                    nc.scalar.activation(
                        ts_t[:, :cw], pt[:, :cw], relu, bias=b1s[:, j : j + 1]
                    )
                    nc.gpsimd.tensor_copy(hh[:, j, :cw], ts_t[:, :cw])
                    nc.gpsimd.tensor_sub(hl[:, j, :cw], ts_t[:, :cw], hh[:, j, :cw])
                return hh, hl

            def g2(ci, e, goff, cw, hh, hl, tail=False):
                w2h, w2l = wts[e][3], wts[e][4]
                HB = DM // 2  # dm tiles per combined out-write
                for dm in range(DM):
                    pt2 = ps2.tile([P, FD], f32, tag="ps2", name=f"ps2_{ci}_{dm}")
                    dms = slice(dm * P, (dm + 1) * P)
                    n = 0
                    for src_w, src_h in ((w2h, hh), (w2l, hh), (w2h, hl)):
                        for kp in range(0, KSLOT, 2):
                            nc.tensor.matmul(
                                pt2[:, :cw],
                                src_w[:, kp : kp + 2, dms],
                                src_h[:, kp : kp + 2, :cw],
                                start=(n == 0),
                                stop=(n == 5),
                                perf_mode=DRM,
                            )
                            n += 1
                    half, hj = divmod(dm, HB)
                    if hj == 0:
                        ob = op.tile(
                            [P, HB, FD], mybir.dt.bfloat16,
                            tag=f"ob{half}", name=f"ob_{ci}_{half}",
                        )
                    if tail and dm % 2 == 0:
                        # nothing else left for ACT at the end of the program:
                        # split the final evictions to halve the drain
                        nc.scalar.copy(ob[:, hj, :cw], pt2[:, :cw])
                    else:
                        nc.vector.tensor_copy(ob[:, hj, :cw], pt2[:, :cw])
                    if hj == HB - 1:
                        # one combined write per 4 dm tiles: DMA-queue pushes
                        # cost ~1us of SEQ time each, so batch them
                        dst = ye_d[
                            half * HB : (half + 1) * HB, :, goff : goff + cw
                        ].transpose([1, 0, 2])
                        eng = nc.scalar if (tail and half == 0) else nc.sync
                        eng.dma_start(dst, ob[:, :, :cw])

            # Software pipeline: emit G1(i) then G2(i-1), so the hid hi/lo
            # eviction chain (ACT -> gpsimd -> DVE) of chunk i completes while
            # the PE runs G1(i+1); G2(i) then starts with all inputs ready.
            prev = None  # (ci, e, goff, cw, hh, hl)
            for ci, (e, goff, cw) in enumerate(work):
                xhc, xlc = x_pre.pop(ci) if ci in x_pre else load_x(ci, goff, cw)
                # prefetch next chunk's x and (at this expert's 2nd chunk or
                # at its last chunk) the next expert's weights
                if ci + 1 < len(work):
                    ne, ngoff, ncw = work[ci + 1]
                    x_pre[ci + 1] = load_x(ci + 1, ngoff, ncw)
                    if ne not in wts:
                        wts[ne] = load_w1(ne) + load_w2(ne)
                hh, hl = g1(ci, e, cw, xhc, xlc)
                if prev is not None:
                    g2(*prev, tail=(ci == len(work) - 1))
                prev = (ci, e, goff, cw, hh, hl)
            g2(*prev, tail=True)

    nc.compile()
    _compiled[key] = nc
    return nc


def _prep_weights(W1, b1, W2):
    """Quantize + relayout weights; returns per-core input maps (cached)."""
    W1 = np.asarray(W1, dtype=np.float32)
    W2 = np.asarray(W2, dtype=np.float32)
    b1 = np.asarray(b1, dtype=np.float32)
    # [E, P(dpart), KH, KD, P(hcol)]
    w1t = np.ascontiguousarray(
        W1.reshape(E, KD, P, KH, P).transpose(0, 2, 3, 1, 4)
    )
    w1h = w1t.astype(E4)
    w1l = (w1t - w1h.astype(np.float32)).astype(E5)
    # [E, P(hpart), KH, D]
    w2t = np.ascontiguousarray(W2.reshape(E, KH, P, D).transpose(0, 2, 1, 3))
    w2h = w2t.astype(E4)
    w2l = (w2t - w2h.astype(np.float32)).astype(E5)
    # [E, P, KH]
    b1t = np.ascontiguousarray(b1.reshape(E, KH, P).transpose(0, 2, 1))
    per_core = []
    for c in range(NCORES):
        js = slice(KSLOT * c, KSLOT * (c + 1))
        per_core.append(
            {
                "w1h": np.ascontiguousarray(w1h[:, :, js]),
                "w1l": np.ascontiguousarray(w1l[:, :, js]),
                "w2h": np.ascontiguousarray(w2h[:, :, js]),
                "w2l": np.ascontiguousarray(w2l[:, :, js]),
                "b1": np.ascontiguousarray(b1t[:, :, js]),
            }
        )
    return per_core


def kernel(x, Wg, bg, W1, b1, W2, b2):
    global LAST_RESULT
    x = np.ascontiguousarray(x, dtype=np.float32)
    B, S, d = x.shape
    assert d == D
    T = B * S
    xf = x.reshape(T, d)

    # ---- Host gating/routing (fp64; tie margins far above fp32 noise) ----
    logits = xf.astype(np.float64) @ Wg.astype(np.float64) + bg.astype(np.float64)
    mx = logits.max(axis=1, keepdims=True)
    ex = np.exp(logits - mx)
    probs = ex / ex.sum(axis=1, keepdims=True)
    order = np.argsort(-logits, axis=1, kind="stable")  # ties -> lower index
    top = order[:, :TOPK]  # [T, 2]
    gsel = np.take_along_axis(probs, top, axis=1).astype(np.float32)

    toks, gates = [], []
    for e in range(E):
        pos = top == e  # [T, 2]
        sel = pos.any(axis=1)
        toks.append(np.nonzero(sel)[0])
        gates.append((gsel * pos).sum(axis=1)[sel].astype(np.float32))
    counts = [len(t) for t in toks]
    TT = int(sum(counts))

    # ---- Gather + quantize x (hi/lo fp8), transposed [P, KD, TT] ----
    xg = xf[np.concatenate(toks)] if TT else np.zeros((0, D), np.float32)
    xt = np.ascontiguousarray(xg.T.reshape(KD, P, TT).transpose(1, 0, 2))
    xh = xt.astype(E4)
    xl = (xt - xh.astype(np.float32)).astype(E5)

    w_maps = _prep_weights(W1, b1, W2)
    nc = _build(counts)

    in_maps = [{"xh": xh, "xl": xl, **w_maps[c]} for c in range(NCORES)]
    res = run_bass_kernel_spmd(nc, in_maps, core_ids=list(range(NCORES)))
    LAST_RESULT = res

    # ---- Combine: sum per-core partials, apply gates + b2, scatter ----
    acc = np.zeros((DM, P, TT), np.float32)
    for c in range(NCORES):
        acc += res.results[c]["ye"].astype(np.float32)
    ye = acc.reshape(D, TT)  # ye[d, t]
    b2f = np.asarray(b2, dtype=np.float32)
    out = np.zeros((T, D), np.float32)
    for e in range(E):
        if counts[e] == 0:
            continue
        blk = ye[:, offs_e(counts, e) : offs_e(counts, e) + counts[e]].T
        out[toks[e]] += gates[e][:, None] * (blk + b2f[e])
    return out.reshape(B, S, D)


def offs_e(counts, e):
    return int(np.sum(counts[:e]))
